# revision 8
# baseline (speedup 1.0000x reference)
"""AttentionFreeTransformer kernel for 8 TRN2 NeuronCores.

Reference computation (B=4, T=4096, D=2048):
    qkv = rmsnorm(x) @ w_qkv.T            # [B, T, 3D]
    q, k, v = split(qkv)
    q = rmsnorm(q); k = rmsnorm(k)
    w = exp(k); kv = w * v
    y = cumsum(kv, T) / (cumsum(w, T) + 1e-6)
    out = (x, sigmoid(q) * y)

Sharding: core = 2*b + h owns batch b, sequence half h (TL=2048 tokens).
Device tensors live transposed [channel partitions, token free] so the
T-cumsum is a DVE tensor_tensor_scan along the free axis; the cross-core
carry (first-half column totals -> second-half core) is the scan's
per-partition `initial`, exchanged with one 16KB pairwise AllReduce.

Schedule (PE never idles between phases; everything else hides under it):
  K phase   tci-outer in two j-halves with resident K weights, chasing the
            xT chunk DMAs so matmuls start ~15us in.  k ssq accumulated
            per chunk; x squares+ssq quads interleaved; k spilled to DRAM.
  V phase   j-outer streamed weights; v=psum*inv_x (DVE), kn=k*inv_k
            (gpsimd), w=exp (ACT, accum), kv=w*v (DVE stt, accum); w/kv
            spilled.  Carry AllReduce at the end, overlapped with Q.
  Q phase   j-outer streamed weights; q chunks copied+squared from psum
            (quad ssq MMs); q spilled.  Scans + ln/exp + y-mul for all 16
            channels interleaved on DVE/ACT/gpsimd under the Q matmuls,
            y spilled, then prefetch q/y reloads for the tail.
  tail      inv_q chain, then per channel sigmoid(q*inv_q)*y, bf16 out.

Algebraic notes:
  - rmsnorm(x)'s per-token scale inv_x factors out of the projection;
    q and k are re-rmsnormed which cancels it, so only v needs inv_x.
  - rsqrt/reciprocal via exp(-0.5*ln(.)) / exp(-ln(.)) on ACT
    (natural_log_exp table set; Rsqrt/Reciprocal ACT funcs banned).
"""

import sys

sys.path.insert(0, "/opt/trn_rl_repo")

import numpy as np
import ml_dtypes

import concourse.bass as bass
import concourse.bacc as bacc_mod
import concourse.mybir as mybir
from concourse.bass import ds, ts
from concourse.tile import TileContext

BF16 = ml_dtypes.bfloat16
F32EPS = float(np.finfo(np.float32).eps)

B, T, D = 4, 4096, 2048
NCORES = 8
TL = T // 2  # tokens per core

AF = mybir.ActivationFunctionType
ALU = mybir.AluOpType


class _Bacc(bacc_mod.Bacc):
    """Bacc whose act-table chooser maps all our funcs to one set.

    Forces Exp/Ln/Square/Copy -> natural_log_exp_and_others and
    Sigmoid -> sigmoid_and_others: 2 ACT_TABLE_LOADs total."""

    def insert_act_table_loads(self):
        from concourse.hw_specs import get_activation_tables
        from concourse.bacc import _bass_rust

        has_activation = any(
            isinstance(i, mybir.InstActivation)
            for b in self.main_func.blocks
            for i in b.instructions
        )
        if not has_activation:
            return
        ours = {AF.Exp, AF.Ln, AF.Square, AF.Copy, AF.Identity, AF.Sigmoid}
        tables = []
        for name, funcs in get_activation_tables(self.m.arch).items():
            if name == "natural_log_exp_and_others":
                tables.append((name, funcs))
            elif name == "sigmoid_and_others":
                tables.append((name, (funcs - ours) | {AF.Sigmoid}))
            else:
                tables.append((name, funcs - ours))
        _bass_rust.insert_act_table_loads(self, tables)


def build_kernel(D_=D, TL_=TL, n_cores=NCORES):
    P = 128
    CH = 512              # token chunk (psum free dim)
    ND = D_ // P          # channel subtiles per projection
    NT = TL_ // CH        # token chunks
    NDH = max(ND // 2, 1) # j-half size for the K phase
    inv_scale = 1.0 / D_

    nc = _Bacc(target_bir_lowering=False, num_devices=n_cores)

    f32 = mybir.dt.float32
    bf16 = mybir.dt.bfloat16

    xT_h = nc.declare_dram_parameter("xT", [P, ND, TL_], bf16, isOutput=False)
    wT_h = nc.declare_dram_parameter("wT", [3 * ND, P, ND, P], bf16, isOutput=False)
    cmask_h = nc.declare_dram_parameter("cmask", [P, 1], f32, isOutput=False)
    smask_h = nc.declare_dram_parameter("smask", [P, 1], f32, isOutput=False)
    out_h = nc.declare_dram_parameter("outT", [ND, P, TL_], bf16, isOutput=True)

    ones_col_h = nc.inline_tensor(np.ones((P, 1), dtype=BF16), name="ones_col")
    ones_row_h = nc.inline_tensor(np.ones((1, P), dtype=BF16), name="ones_row")

    groups = [[i, i + 1] for i in range(0, n_cores, 2)]

    with (
        TileContext(nc) as tc,
        tc.tile_pool(name="const", bufs=1) as const,
        tc.tile_pool(name="wk", bufs=NDH) as wkp,          # resident K weights (half)
        tc.tile_pool(name="wstream", bufs=2) as wstream,   # V/Q streamed weights
        tc.tile_pool(name="chunk", bufs=7) as chunkp,      # [P,CH] bf16 chunks
        tc.tile_pool(name="s16", bufs=13) as s16,          # [P,TL] bf16 scratch
        tc.tile_pool(name="lw32", bufs=2) as lw32,         # [P,TL] f32 scratch
        tc.tile_pool(name="rows", bufs=1) as rows,
        tc.tile_pool(name="mmps", bufs=5, space="PSUM") as mmps,
        tc.tile_pool(name="ssqps", bufs=2, space="PSUM") as ssqps,
        tc.tile_pool(name="repps", bufs=1, space="PSUM") as repps,
        tc.tile_pool(name="spill", bufs=1, space="DRAM") as spill,
    ):
        # ---- constants / resident tiles ----
        ones_col = const.tile([P, 1], bf16, tag="ones_col")
        nc.sync.dma_start(out=ones_col[:], in_=ones_col_h[:])
        ones_row = const.tile([1, P], bf16, tag="ones_row")
        nc.sync.dma_start(out=ones_row[:], in_=ones_row_h[:])
        cmask = const.tile([P, 1], f32, tag="cmask")
        nc.sync.dma_start(out=cmask[:], in_=cmask_h[:])
        smask = const.tile([P, 1], f32, tag="smask")
        nc.sync.dma_start(out=smask[:], in_=smask_h[:])

        eps_b = const.tile([P, 1], f32, tag="eps_b")
        nc.vector.memset(eps_b[:], F32EPS)
        eps6_b = const.tile([P, 1], f32, tag="eps6_b")
        nc.vector.memset(eps6_b[:], 1e-6)

        inv_x = const.tile([P, TL_], bf16, tag="inv_x")
        inv_k = const.tile([P, TL_], bf16, tag="inv_k")
        inv_q = const.tile([P, TL_], bf16, tag="inv_q")
        carry_both = const.tile([P, 2 * ND], f32, tag="carry_both")
        carry_use = const.tile([P, 2 * ND], f32, tag="carry_use")

        # xT as NT chunk tiles so matmuls chase the load
        xT_c = [
            const.tile([P, ND, CH], bf16, tag=f"xT{tci}", name=f"xT_c{tci}")
            for tci in range(NT)
        ]

        # ---- DRAM spill arrays ----
        k_sp = spill.tile([ND, P, TL_], bf16, tag="k_sp")
        q_sp = spill.tile([ND, P, TL_], bf16, tag="q_sp")
        w_sp = spill.tile([ND, P, TL_], bf16, tag="w_sp")
        kv_sp = spill.tile([ND, P, TL_], bf16, tag="kv_sp")
        y_sp = spill.tile([ND, P, TL_], bf16, tag="y_sp")
        cc_in = spill.tile([P, 2 * ND], f32, tag="cc_in")
        cc_out = spill.tile([P, 2 * ND], f32, tag="cc_out")

        # ---- input DMAs: first K weight block + xT chunk 0 first ----
        wk_sb = {}
        wk_sb[0] = wkp.tile([P, ND, P], bf16, tag="wk", name="wk0")
        nc.sync.dma_start(out=wk_sb[0][:], in_=wT_h[ND + 0])
        nc.sync.dma_start(out=xT_c[0][:], in_=xT_h[:, :, ts(0, CH)])
        for j in range(1, NDH):
            wk_sb[j] = wkp.tile([P, ND, P], bf16, tag="wk", name=f"wk{j}")
            nc.sync.dma_start(out=wk_sb[j][:], in_=wT_h[ND + j])
        for tci in range(1, NT):
            nc.sync.dma_start(out=xT_c[tci][:], in_=xT_h[:, :, ts(tci, CH)])

        # ssq accumulators: one [P,CH] psum tile per projection, row 32*tci
        xssq = ssqps.tile([P, CH], f32, tag="ssq", name="xssq")
        kssq = ssqps.tile([P, CH], f32, tag="ssq", name="kssq")

        def proj_group(wsb, tci, name):
            """One accumulation group: psum[P,CH] = w_blk.T @ xT chunk."""
            pk = mmps.tile([P, CH], f32, tag="mm", name=name)
            for do in range(ND):
                nc.tensor.matmul(
                    out=pk[:],
                    lhsT=wsb[:, do, :],
                    rhs=xT_c[tci][:, do, :],
                    start=(do == 0),
                    stop=(do == ND - 1),
                )
            return pk

        def ssq_mm(ssq_tile, sq_chunk, tci, start, stop):
            # explicit tile_position: auto-derive rejects base partition 96
            nc.tensor.matmul(
                out=ssq_tile[32 * tci : 32 * tci + 1, :],
                lhsT=ones_col[:],
                rhs=sq_chunk[:],
                start=start,
                stop=stop,
                tile_position=(0, 32 * tci),
            )

        def inv_chain(ssq_tile, dest, extra_scale):
            """dest[p,t] = (ssq[t]/D + eps) ** (extra_scale) replicated."""
            row = rows.tile([1, TL_], bf16, tag="row")
            for tci in range(NT):
                nc.scalar.copy(
                    out=row[:, ts(tci, CH)],
                    in_=ssq_tile[32 * tci : 32 * tci + 1, :],
                )
            lnv = lw32.tile([P, TL_], f32, tag="lw")
            for tci in range(NT):
                rep = repps.tile([P, CH], f32, tag="rep", name="rep")
                nc.tensor.matmul(
                    out=rep[:],
                    lhsT=ones_row[:],
                    rhs=row[:, ts(tci, CH)],
                    start=True,
                    stop=True,
                )
                nc.scalar.activation(
                    lnv[:, ts(tci, CH)], rep[:], AF.Ln,
                    bias=eps_b[:], scale=inv_scale,
                )
            nc.scalar.activation(dest[:], lnv[:], AF.Exp, scale=extra_scale)

        # ================= K phase =================
        # two j-halves; tci-outer within each so MMs chase the xT chunks.
        # k ssq: row 32*tci accumulates over all ND j's (across both halves).
        pending_kssq = []  # staggered one group to keep PE in-order happy

        def flush_kssq(n):
            while len(pending_kssq) > n:
                pending_kssq.pop(0)()

        xsq_done = [False] * ND
        for jh in range(2 if ND > 1 else 1):
            j0 = jh * NDH
            for tci in range(NT):
                for jj in range(NDH):
                    j = j0 + jj
                    if j not in wk_sb:
                        wk_sb[j] = wkp.tile([P, ND, P], bf16, tag="wk", name=f"wk{j}")
                        nc.sync.dma_start(out=wk_sb[j][:], in_=wT_h[ND + j])
                    pk = proj_group(wk_sb[j], tci, f"pk{j}_{tci}")
                    ksb = chunkp.tile([P, CH], bf16, tag="ch", name=f"k{j}_{tci}")
                    nc.scalar.copy(out=ksb[:], in_=pk[:])
                    ksq = chunkp.tile([P, CH], bf16, tag="ch", name=f"ksq{j}_{tci}")
                    nc.scalar.activation(ksq[:], pk[:], AF.Square)
                    nc.gpsimd.dma_start(out=k_sp[j, :, ts(tci, CH)], in_=ksb[:])
                    pending_kssq.append(
                        (lambda kq=ksq, tc_=tci, j_=j: ssq_mm(
                            kssq, kq, tc_, start=(j_ == 0), stop=(j_ == ND - 1)))
                    )
                    flush_kssq(1)
        # x ssq: full-tile squares + quad MMs (emitted after K proj loops;
        # ACT/PE have slack and deps are long ready)
        for do in range(ND):
            sq = s16.tile([P, TL_], bf16, tag="s16", name=f"xsq{do}")
            for tci in range(NT):
                nc.scalar.activation(
                    sq[:, ts(tci, CH)], xT_c[tci][:, do, :], AF.Square
                )
            for tci in range(NT):
                ssq_mm(xssq, sq[:, ts(tci, CH)], tci,
                       start=(do == 0), stop=(do == ND - 1))
        flush_kssq(0)

        inv_chain(kssq, inv_k, -0.5)
        inv_chain(xssq, inv_x, -0.5)

        # ================= V phase =================
        kc_tiles = {}
        for c in range(min(2, ND)):  # prefetch k reloads
            kc_tiles[c] = s16.tile([P, TL_], bf16, tag="s16", name=f"kc{c}")
            nc.sync.dma_start(out=kc_tiles[c][:], in_=k_sp[c])

        for c in range(ND):
            wv = wstream.tile([P, ND, P], bf16, tag="wv", name=f"wv{c}")
            nc.sync.dma_start(out=wv[:], in_=wT_h[2 * ND + c])
            vsb = s16.tile([P, TL_], bf16, tag="s16", name=f"v{c}")
            for tci in range(NT):
                pv = proj_group(wv, tci, f"pv{c}_{tci}")
                nc.vector.tensor_mul(
                    out=vsb[:, ts(tci, CH)], in0=pv[:], in1=inv_x[:, ts(tci, CH)]
                )
            if c + 2 < ND:
                kc_tiles[c + 2] = s16.tile([P, TL_], bf16, tag="s16",
                                           name=f"kc{c + 2}")
                nc.sync.dma_start(out=kc_tiles[c + 2][:], in_=k_sp[c + 2])
            kn = s16.tile([P, TL_], bf16, tag="s16", name=f"kn{c}")
            nc.gpsimd.tensor_mul(out=kn[:], in0=kc_tiles[c][:], in1=inv_k[:])
            wc = s16.tile([P, TL_], bf16, tag="s16", name=f"w{c}")
            nc.scalar.activation(
                wc[:], kn[:], AF.Exp, accum_out=carry_both[:, c : c + 1]
            )
            kvc = s16.tile([P, TL_], bf16, tag="s16", name=f"kv{c}")
            nc.vector.scalar_tensor_tensor(
                out=kvc[:], in0=wc[:], scalar=1.0, in1=vsb[:],
                op0=ALU.mult, op1=ALU.mult,
                accum_out=carry_both[:, ND + c : ND + c + 1],
            )
            nc.gpsimd.dma_start(out=w_sp[c], in_=wc[:])
            nc.gpsimd.dma_start(out=kv_sp[c], in_=kvc[:])

        # ---- carry exchange (overlaps with Q phase matmuls) ----
        snd = lw32.tile([P, 2 * ND], f32, tag="lw", name="snd")
        nc.vector.tensor_scalar_mul(snd[:], carry_both[:], smask[:])
        nc.gpsimd.dma_start(out=cc_in[:], in_=snd[:])
        nc.gpsimd.collective_compute(
            "AllReduce",
            ALU.add,
            replica_groups=groups,
            ins=[cc_in[:]],
            outs=[cc_out[:]],
        )
        rcv = lw32.tile([P, 2 * ND], f32, tag="lw", name="rcv")
        nc.sync.dma_start(out=rcv[:], in_=cc_out[:])
        nc.vector.tensor_scalar_mul(carry_use[:], rcv[:], cmask[:])

        # ================= Q phase (+ scans interleaved) =================
        qssq = ssqps.tile([P, CH], f32, tag="ssq", name="qssq")
        SCAN_DELAY = 2  # emit scan-consumer ACT/gpsimd ops a few groups late

        scan_tiles = {}

        def emit_scan(c):
            """Reload w/kv, scan both, compute y, spill y."""
            wld = s16.tile([P, TL_], bf16, tag="s16", name=f"wld{c}")
            nc.sync.dma_start(out=wld[:], in_=w_sp[c])
            kvld = s16.tile([P, TL_], bf16, tag="s16", name=f"kvld{c}")
            nc.sync.dma_start(out=kvld[:], in_=kv_sp[c])
            wcum = s16.tile([P, TL_], bf16, tag="s16", name=f"wcum{c}")
            nc.vector.tensor_tensor_scan(
                out=wcum[:], data0=wld[:], data1=wld[:],
                initial=carry_use[:, c : c + 1],
                op0=ALU.add, op1=ALU.bypass,
            )
            kvcum = s16.tile([P, TL_], bf16, tag="s16", name=f"kvcum{c}")
            nc.vector.tensor_tensor_scan(
                out=kvcum[:], data0=kvld[:], data1=kvld[:],
                initial=carry_use[:, ND + c : ND + c + 1],
                op0=ALU.add, op1=ALU.bypass,
            )
            scan_tiles[c] = (wcum, kvcum)

        def emit_y(c):
            wcum, kvcum = scan_tiles.pop(c)
            lw = lw32.tile([P, TL_], f32, tag="lw", name=f"lw{c}")
            nc.scalar.activation(lw[:], wcum[:], AF.Ln, bias=eps6_b[:])
            rw = s16.tile([P, TL_], bf16, tag="s16", name=f"rw{c}")
            nc.scalar.activation(rw[:], lw[:], AF.Exp, scale=-1.0)
            yc = s16.tile([P, TL_], bf16, tag="s16", name=f"y{c}")
            nc.gpsimd.tensor_mul(out=yc[:], in0=kvcum[:], in1=rw[:])
            nc.gpsimd.dma_start(out=y_sp[c], in_=yc[:])

        for j in range(ND):
            wq = wstream.tile([P, ND, P], bf16, tag="wq", name=f"wq{j}")
            nc.sync.dma_start(out=wq[:], in_=wT_h[0 + j])
            qsb = s16.tile([P, TL_], bf16, tag="s16", name=f"q{j}")
            sqs = []
            for tci in range(NT):
                pq = proj_group(wq, tci, f"pq{j}_{tci}")
                nc.scalar.copy(out=qsb[:, ts(tci, CH)], in_=pq[:])
                qsq = chunkp.tile([P, CH], bf16, tag="ch", name=f"qsq{j}_{tci}")
                nc.scalar.activation(qsq[:], pq[:], AF.Square)
                sqs.append(qsq)
            # quad ssq MMs: 4 adjacent MMs hit col-groups 0/32/64/96
            for tci in range(NT):
                ssq_mm(qssq, sqs[tci], tci, start=(j == 0), stop=(j == ND - 1))
            nc.gpsimd.dma_start(out=q_sp[j], in_=qsb[:])
            # interleaved scan pipeline
            if j >= 1 and (j - 1) < ND:
                emit_scan(j - 1)
            if j >= SCAN_DELAY + 1:
                emit_y(j - SCAN_DELAY - 1)
        emit_scan(ND - 1)
        for c in range(max(ND - SCAN_DELAY - 1, 0), ND):
            emit_y(c)

        inv_chain(qssq, inv_q, -0.5)

        # ================= tail =================
        ql_tiles, yl_tiles = {}, {}
        for c in range(min(2, ND)):
            ql_tiles[c] = s16.tile([P, TL_], bf16, tag="s16", name=f"ql{c}")
            nc.sync.dma_start(out=ql_tiles[c][:], in_=q_sp[c])
            yl_tiles[c] = s16.tile([P, TL_], bf16, tag="s16", name=f"yl{c}")
            nc.sync.dma_start(out=yl_tiles[c][:], in_=y_sp[c])
        for c in range(ND):
            if c + 2 < ND:
                ql_tiles[c + 2] = s16.tile([P, TL_], bf16, tag="s16",
                                           name=f"ql{c + 2}")
                nc.sync.dma_start(out=ql_tiles[c + 2][:], in_=q_sp[c + 2])
                yl_tiles[c + 2] = s16.tile([P, TL_], bf16, tag="s16",
                                           name=f"yl{c + 2}")
                nc.sync.dma_start(out=yl_tiles[c + 2][:], in_=y_sp[c + 2])
            qi = s16.tile([P, TL_], bf16, tag="s16", name=f"qi{c}")
            qi_eng = nc.gpsimd if c % 2 == 0 else nc.vector
            qi_eng.tensor_mul(out=qi[:], in0=ql_tiles.pop(c)[:], in1=inv_q[:])
            sg = s16.tile([P, TL_], bf16, tag="s16", name=f"sg{c}")
            nc.scalar.activation(sg[:], qi[:], AF.Sigmoid)
            outc = s16.tile([P, TL_], bf16, tag="s16", name=f"out{c}")
            nc.vector.tensor_mul(out=outc[:], in0=sg[:], in1=yl_tiles.pop(c)[:])
            nc.gpsimd.dma_start(out=out_h[c], in_=outc[:])

    nc.finalize()
    return nc


def make_in_maps(x, w_qkv, D_=D, TL_=TL, n_cores=NCORES):
    """Host-side shard + layout prep. Returns per-core input dicts."""
    P = 128
    ND = D_ // P
    E = w_qkv.shape[0]
    n_eblk = E // P
    b_count = x.shape[0]
    halves = n_cores // b_count

    # wT tiled: [e_blk, p, do, pe] with wtile[blk, p, do, e] = w_qkv[blk*128+e, do*128+p]
    wt = (
        np.ascontiguousarray(
            w_qkv.T.reshape(ND, P, n_eblk, P).transpose(2, 1, 0, 3)
        ).astype(BF16)
    )

    in_maps = []
    for core in range(n_cores):
        b, h = divmod(core, halves)
        shard = x[b, h * TL_ : (h + 1) * TL_, :]  # [TL, D]
        xt = np.ascontiguousarray(
            shard.T.reshape(ND, P, TL_).transpose(1, 0, 2)
        ).astype(BF16)
        odd = float(h % 2 == 1)
        in_maps.append(
            {
                "xT": xt,
                "wT": wt,
                "cmask": np.full((P, 1), odd, dtype=np.float32),
                "smask": np.full((P, 1), 1.0 - odd, dtype=np.float32),
            }
        )
    return in_maps


def assemble_output(results, x, D_=D, TL_=TL, n_cores=NCORES):
    b_count = x.shape[0]
    halves = n_cores // b_count
    out2 = np.empty((b_count, halves * TL_, D_), dtype=np.float32)
    for core in range(n_cores):
        b, h = divmod(core, halves)
        outT = np.asarray(results[core]["outT"]).astype(np.float32).reshape(D_, TL_)
        out2[b, h * TL_ : (h + 1) * TL_, :] = outT.T
    return out2


_CACHED_NC = None


def kernel(x, w_qkv):
    global _CACHED_NC
    from concourse.bass_utils import run_bass_kernel_spmd

    x = np.asarray(x, dtype=np.float32)
    w_qkv = np.asarray(w_qkv, dtype=np.float32)

    if _CACHED_NC is None:
        _CACHED_NC = build_kernel()
    in_maps = make_in_maps(x, w_qkv)
    res = run_bass_kernel_spmd(_CACHED_NC, in_maps, core_ids=list(range(NCORES)))
    out2 = assemble_output(res.results, x)
    return (x, out2)


# revision 16
# speedup vs baseline: 1.2244x; 1.2244x over previous
"""AttentionFreeTransformer kernel for 8 TRN2 NeuronCores.

Reference computation (B=4, T=4096, D=2048):
    qkv = rmsnorm(x) @ w_qkv.T            # [B, T, 3D]
    q, k, v = split(qkv)
    q = rmsnorm(q); k = rmsnorm(k)
    w = exp(k); kv = w * v
    y = cumsum(kv, T) / (cumsum(w, T) + 1e-6)
    out = (x, sigmoid(q) * y)

Sharding: core = 2*b + h owns batch b, sequence half h (TL=2048 tokens).
Device tensors live transposed [channel partitions, token free] so the
T-cumsum is a DVE tensor_tensor_scan along the free axis; the cross-core
carry (first-half column totals -> second-half core) is the scan's
per-partition `initial`, exchanged with one 16KB pairwise AllReduce.

Schedule (PE never idles between phases; everything else hides under it):
  K phase   tci-outer in two j-halves with resident K weights, chasing the
            xT chunk DMAs so matmuls start ~15us in.  k ssq accumulated
            per chunk; x squares+ssq quads interleaved; k spilled to DRAM.
  V phase   j-outer streamed weights; v=psum*inv_x (DVE), kn=k*inv_k
            (gpsimd), w=exp (ACT, accum), kv=w*v (DVE stt, accum); w/kv
            spilled.  Carry AllReduce at the end, overlapped with Q.
  Q phase   j-outer streamed weights; q chunks copied+squared from psum
            (quad ssq MMs); q spilled.  Scans + ln/exp + y-mul for all 16
            channels interleaved on DVE/ACT/gpsimd under the Q matmuls,
            y spilled, then prefetch q/y reloads for the tail.
  tail      inv_q chain, then per channel sigmoid(q*inv_q)*y, bf16 out.

Algebraic notes:
  - rmsnorm(x)'s per-token scale inv_x factors out of the projection;
    q and k are re-rmsnormed which cancels it, so only v needs inv_x.
  - rsqrt/reciprocal via exp(-0.5*ln(.)) / exp(-ln(.)) on ACT
    (natural_log_exp table set; Rsqrt/Reciprocal ACT funcs banned).
"""

import sys

sys.path.insert(0, "/opt/trn_rl_repo")

import numpy as np
import ml_dtypes

import concourse.bass as bass
import concourse.bacc as bacc_mod
import concourse.mybir as mybir
from concourse.bass import ds, ts
from concourse.tile import TileContext

BF16 = ml_dtypes.bfloat16
F32EPS = float(np.finfo(np.float32).eps)

B, T, D = 4, 4096, 2048
NCORES = 8
TL = T // 2  # tokens per core

AF = mybir.ActivationFunctionType
ALU = mybir.AluOpType


class _Bacc(bacc_mod.Bacc):
    """Bacc whose act-table chooser maps all our funcs to one set.

    Forces Exp/Ln/Square/Copy -> natural_log_exp_and_others and
    Sigmoid -> sigmoid_and_others: 2 ACT_TABLE_LOADs total."""

    def insert_act_table_loads(self):
        from concourse.hw_specs import get_activation_tables
        from concourse.bacc import _bass_rust

        has_activation = any(
            isinstance(i, mybir.InstActivation)
            for b in self.main_func.blocks
            for i in b.instructions
        )
        if not has_activation:
            return
        ours = {AF.Exp, AF.Ln, AF.Square, AF.Copy, AF.Identity, AF.Sigmoid}
        tables = []
        for name, funcs in get_activation_tables(self.m.arch).items():
            if name == "natural_log_exp_and_others":
                tables.append((name, funcs))
            elif name == "sigmoid_and_others":
                tables.append((name, (funcs - ours) | {AF.Sigmoid}))
            else:
                tables.append((name, funcs - ours))
        _bass_rust.insert_act_table_loads(self, tables)


def build_kernel(D_=D, TL_=TL, n_cores=NCORES):
    P = 128
    CH = 512              # token chunk (psum free dim)
    ND = D_ // P          # channel subtiles per projection
    NT = TL_ // CH        # token chunks
    NDH = max(ND // 2, 1) # j-half size for the K phase
    inv_scale = 1.0 / D_

    nc = _Bacc(target_bir_lowering=False, num_devices=n_cores)

    f32 = mybir.dt.float32
    bf16 = mybir.dt.bfloat16

    xT_h = nc.declare_dram_parameter("xT", [P, ND, TL_], bf16, isOutput=False)
    wT_h = nc.declare_dram_parameter("wT", [3 * ND, P, ND, P], bf16, isOutput=False)
    cmask_h = nc.declare_dram_parameter("cmask", [P, 1], f32, isOutput=False)
    smask_h = nc.declare_dram_parameter("smask", [P, 1], f32, isOutput=False)
    out_h = nc.declare_dram_parameter("outT", [ND, P, TL_], bf16, isOutput=True)

    ones_col_h = nc.inline_tensor(np.ones((P, 1), dtype=BF16), name="ones_col")
    ones_row_h = nc.inline_tensor(np.ones((1, P), dtype=BF16), name="ones_row")

    groups = [[i, i + 1] for i in range(0, n_cores, 2)]

    with (
        TileContext(nc) as tc,
        tc.tile_pool(name="const", bufs=1) as const,
        tc.tile_pool(name="wk", bufs=NDH) as wkp,          # resident K weights (half)
        tc.tile_pool(name="wstream", bufs=3) as wstream,   # V/Q streamed weights
        tc.tile_pool(name="chunk", bufs=11) as chunkp,     # [P,CH] bf16 chunks
        tc.tile_pool(name="s16", bufs=12) as s16,          # [P,TL] bf16 scratch
        tc.tile_pool(name="lw32", bufs=1) as lw32,         # [P,TL] f32 scratch
        tc.tile_pool(name="rows", bufs=1) as rows,
        tc.tile_pool(name="mmps", bufs=5, space="PSUM") as mmps,
        tc.tile_pool(name="ssqps", bufs=2, space="PSUM") as ssqps,
        tc.tile_pool(name="repps", bufs=1, space="PSUM") as repps,
        tc.tile_pool(name="spill", bufs=1, space="DRAM") as spill,
    ):
        # ---- constants / resident tiles ----
        ones_col = const.tile([P, 1], bf16, tag="ones_col")
        nc.sync.dma_start(out=ones_col[:], in_=ones_col_h[:])
        ones_row = const.tile([1, P], bf16, tag="ones_row")
        nc.sync.dma_start(out=ones_row[:], in_=ones_row_h[:])
        cmask = const.tile([P, 1], f32, tag="cmask")
        nc.sync.dma_start(out=cmask[:], in_=cmask_h[:])
        smask = const.tile([P, 1], f32, tag="smask")
        nc.sync.dma_start(out=smask[:], in_=smask_h[:])

        eps_b = const.tile([P, 1], f32, tag="eps_b")
        nc.vector.memset(eps_b[:], F32EPS)
        eps6_b = const.tile([P, 1], f32, tag="eps6_b")
        nc.vector.memset(eps6_b[:], 1e-6)

        inv_x = const.tile([P, TL_], bf16, tag="inv_x")
        inv_k = const.tile([P, TL_], bf16, tag="inv_k")
        inv_q = const.tile([P, TL_], bf16, tag="inv_q")
        carry_both = const.tile([P, 2 * ND], f32, tag="carry_both")
        carry_use = const.tile([P, 2 * ND], f32, tag="carry_use")

        # xT as NT chunk tiles so matmuls chase the load
        xT_c = [
            const.tile([P, ND, CH], bf16, tag=f"xT{tci}", name=f"xT_c{tci}")
            for tci in range(NT)
        ]

        # ---- DRAM spill arrays ----
        k_sp = spill.tile([ND, P, TL_], bf16, tag="k_sp")
        q_sp = spill.tile([ND, P, TL_], bf16, tag="q_sp")
        w_sp = spill.tile([ND, P, TL_], bf16, tag="w_sp")
        kv_sp = spill.tile([ND, P, TL_], bf16, tag="kv_sp")
        y_sp = spill.tile([ND, P, TL_], bf16, tag="y_sp")
        cc_in = spill.tile([P, 2 * ND], f32, tag="cc_in")
        cc_out = spill.tile([P, 2 * ND], f32, tag="cc_out")

        # ---- input DMAs: first K weight block + xT chunk 0 first ----
        wk_sb = {}
        wk_sb[0] = wkp.tile([P, ND, P], bf16, tag="wk", name="wk0")
        nc.scalar.dma_start(out=wk_sb[0][:], in_=wT_h[ND + 0])
        nc.sync.dma_start(out=xT_c[0][:], in_=xT_h[:, :, ts(0, CH)])
        for j in range(1, NDH):
            wk_sb[j] = wkp.tile([P, ND, P], bf16, tag="wk", name=f"wk{j}")
            nc.sync.dma_start(out=wk_sb[j][:], in_=wT_h[ND + j])
        for tci in range(1, NT):
            nc.sync.dma_start(out=xT_c[tci][:], in_=xT_h[:, :, ts(tci, CH)])

        # ssq accumulators: one [P,CH] psum tile per projection, row 32*tci
        xssq = ssqps.tile([P, CH], f32, tag="ssq", name="xssq")
        kssq = ssqps.tile([P, CH], f32, tag="ssq", name="kssq")

        def proj_group(wsb, tci, name):
            """One accumulation group: psum[P,CH] = w_blk.T @ xT chunk."""
            pk = mmps.tile([P, CH], f32, tag="mm", name=name)
            for do in range(ND):
                nc.tensor.matmul(
                    out=pk[:],
                    lhsT=wsb[:, do, :],
                    rhs=xT_c[tci][:, do, :],
                    start=(do == 0),
                    stop=(do == ND - 1),
                )
            return pk

        def ssq_mm(ssq_tile, sq_chunk, tci, start, stop):
            # explicit tile_position: auto-derive rejects base partition 96
            nc.tensor.matmul(
                out=ssq_tile[32 * tci : 32 * tci + 1, :],
                lhsT=ones_col[:],
                rhs=sq_chunk[:],
                start=start,
                stop=stop,
                tile_position=(0, 32 * tci),
            )

        def inv_chain(ssq_tile, dest, extra_scale):
            """dest[p,t] = (ssq[t]/D + eps) ** (extra_scale) replicated."""
            row = rows.tile([1, TL_], bf16, tag="row")
            for tci in range(NT):
                nc.scalar.copy(
                    out=row[:, ts(tci, CH)],
                    in_=ssq_tile[32 * tci : 32 * tci + 1, :],
                )
            lnv = lw32.tile([P, TL_], f32, tag="lw")
            for tci in range(NT):
                rep = repps.tile([P, CH], f32, tag="rep", name="rep")
                nc.tensor.matmul(
                    out=rep[:],
                    lhsT=ones_row[:],
                    rhs=row[:, ts(tci, CH)],
                    start=True,
                    stop=True,
                )
                nc.scalar.activation(
                    lnv[:, ts(tci, CH)], rep[:], AF.Ln,
                    bias=eps_b[:], scale=inv_scale,
                )
            nc.scalar.activation(dest[:], lnv[:], AF.Exp, scale=extra_scale)

        # ================= K phase =================
        # two j-halves; tci-outer within each so MMs chase the xT chunks.
        # k ssq: row 32*tci accumulates over all ND j's (across both halves).
        pending_kssq = []  # staggered one group to keep PE in-order happy

        def flush_kssq(n):
            while len(pending_kssq) > n:
                pending_kssq.pop(0)()

        xsq_done = [False] * ND
        for jh in range(2 if ND > 1 else 1):
            j0 = jh * NDH
            for tci in range(NT):
                for jj in range(NDH):
                    j = j0 + jj
                    if j not in wk_sb:
                        wk_sb[j] = wkp.tile([P, ND, P], bf16, tag="wk", name=f"wk{j}")
                        nc.sync.dma_start(out=wk_sb[j][:], in_=wT_h[ND + j])
                    pk = proj_group(wk_sb[j], tci, f"pk{j}_{tci}")
                    ksb = chunkp.tile([P, CH], bf16, tag="ch", name=f"k{j}_{tci}")
                    nc.scalar.copy(out=ksb[:], in_=pk[:])
                    ksq = chunkp.tile([P, CH], bf16, tag="ch", name=f"ksq{j}_{tci}")
                    nc.vector.tensor_mul(out=ksq[:], in0=ksb[:], in1=ksb[:])
                    nc.gpsimd.dma_start(out=k_sp[j, :, ts(tci, CH)], in_=ksb[:])
                    pending_kssq.append(
                        (lambda kq=ksq, tc_=tci, j_=j: ssq_mm(
                            kssq, kq, tc_, start=(j_ == 0), stop=(j_ == ND - 1)))
                    )
                    flush_kssq(2)
        # x ssq: DVE squares + quad MMs (emitted after K proj loops;
        # DVE has slack in K and deps are long ready)
        for do in range(ND):
            sq = s16.tile([P, TL_], bf16, tag="s16", name=f"xsq{do}")
            for tci in range(NT):
                nc.vector.tensor_mul(
                    out=sq[:, ts(tci, CH)],
                    in0=xT_c[tci][:, do, :], in1=xT_c[tci][:, do, :],
                )
            for tci in range(NT):
                ssq_mm(xssq, sq[:, ts(tci, CH)], tci,
                       start=(do == 0), stop=(do == ND - 1))
        flush_kssq(0)

        inv_chain(kssq, inv_k, -0.5)
        inv_chain(xssq, inv_x, -0.5)

        # ================= V phase =================
        kc_tiles = {}
        for c in range(min(2, ND)):  # prefetch k reloads
            kc_tiles[c] = s16.tile([P, TL_], bf16, tag="s16", name=f"kc{c}")
            nc.sync.dma_start(out=kc_tiles[c][:], in_=k_sp[c])

        for c in range(ND):
            wv = wstream.tile([P, ND, P], bf16, tag="wv", name=f"wv{c}")
            nc.sync.dma_start(out=wv[:], in_=wT_h[2 * ND + c])
            vsb = s16.tile([P, TL_], bf16, tag="s16", name=f"v{c}")
            for tci in range(NT):
                pv = proj_group(wv, tci, f"pv{c}_{tci}")
                nc.vector.tensor_mul(
                    out=vsb[:, ts(tci, CH)], in0=pv[:], in1=inv_x[:, ts(tci, CH)]
                )
            if c + 2 < ND:
                kc_tiles[c + 2] = s16.tile([P, TL_], bf16, tag="s16",
                                           name=f"kc{c + 2}")
                nc.sync.dma_start(out=kc_tiles[c + 2][:], in_=k_sp[c + 2])
            kn = s16.tile([P, TL_], bf16, tag="s16", name=f"kn{c}")
            nc.gpsimd.tensor_mul(out=kn[:], in0=kc_tiles[c][:], in1=inv_k[:])
            wc = s16.tile([P, TL_], bf16, tag="s16", name=f"w{c}")
            nc.scalar.activation(
                wc[:], kn[:], AF.Exp, accum_out=carry_both[:, c : c + 1]
            )
            kvc = s16.tile([P, TL_], bf16, tag="s16", name=f"kv{c}")
            nc.vector.scalar_tensor_tensor(
                out=kvc[:], in0=wc[:], scalar=1.0, in1=vsb[:],
                op0=ALU.mult, op1=ALU.mult,
                accum_out=carry_both[:, ND + c : ND + c + 1],
            )
            nc.gpsimd.dma_start(out=w_sp[c], in_=wc[:])
            nc.gpsimd.dma_start(out=kv_sp[c], in_=kvc[:])

        # ---- carry exchange (overlaps with Q phase matmuls) ----
        snd = lw32.tile([P, 2 * ND], f32, tag="lw", name="snd")
        nc.vector.tensor_scalar_mul(snd[:], carry_both[:], smask[:])
        nc.gpsimd.dma_start(out=cc_in[:], in_=snd[:])
        nc.gpsimd.collective_compute(
            "AllReduce",
            ALU.add,
            replica_groups=groups,
            ins=[cc_in[:]],
            outs=[cc_out[:]],
        )
        rcv = lw32.tile([P, 2 * ND], f32, tag="lw", name="rcv")
        nc.sync.dma_start(out=rcv[:], in_=cc_out[:])
        nc.vector.tensor_scalar_mul(carry_use[:], rcv[:], cmask[:])

        # ================= Q phase (+ scans interleaved) =================
        qssq = ssqps.tile([P, CH], f32, tag="ssq", name="qssq")
        SCAN_DELAY = 1  # groups between a channel's scan and its y compute

        scan_tiles = {}

        def emit_scan(c):
            """Reload w/kv, scan both, compute y, spill y."""
            wld = s16.tile([P, TL_], bf16, tag="s16", name=f"wld{c}")
            nc.sync.dma_start(out=wld[:], in_=w_sp[c])
            kvld = s16.tile([P, TL_], bf16, tag="s16", name=f"kvld{c}")
            nc.sync.dma_start(out=kvld[:], in_=kv_sp[c])
            wcum = s16.tile([P, TL_], bf16, tag="s16", name=f"wcum{c}")
            nc.vector.tensor_tensor_scan(
                out=wcum[:], data0=wld[:], data1=wld[:],
                initial=carry_use[:, c : c + 1],
                op0=ALU.add, op1=ALU.bypass,
            )
            kvcum = s16.tile([P, TL_], bf16, tag="s16", name=f"kvcum{c}")
            nc.vector.tensor_tensor_scan(
                out=kvcum[:], data0=kvld[:], data1=kvld[:],
                initial=carry_use[:, ND + c : ND + c + 1],
                op0=ALU.add, op1=ALU.bypass,
            )
            scan_tiles[c] = (wcum, kvcum)

        def emit_y(c):
            wcum, kvcum = scan_tiles.pop(c)
            lw = lw32.tile([P, TL_], f32, tag="lw", name=f"lw{c}")
            nc.scalar.activation(lw[:], wcum[:], AF.Ln, bias=eps6_b[:])
            rw = s16.tile([P, TL_], bf16, tag="s16", name=f"rw{c}")
            nc.scalar.activation(rw[:], lw[:], AF.Exp, scale=-1.0)
            yc = s16.tile([P, TL_], bf16, tag="s16", name=f"y{c}")
            nc.gpsimd.tensor_mul(out=yc[:], in0=kvcum[:], in1=rw[:])
            nc.gpsimd.dma_start(out=y_sp[c], in_=yc[:])

        pending_qssq = []
        for j in range(ND):
            wq = wstream.tile([P, ND, P], bf16, tag="wq", name=f"wq{j}")
            nc.sync.dma_start(out=wq[:], in_=wT_h[0 + j])
            qsb = s16.tile([P, TL_], bf16, tag="s16", name=f"q{j}")
            sqs = []
            for tci in range(NT):
                pq = proj_group(wq, tci, f"pq{j}_{tci}")
                nc.scalar.copy(out=qsb[:, ts(tci, CH)], in_=pq[:])
                qsq = chunkp.tile([P, CH], bf16, tag="ch", name=f"qsq{j}_{tci}")
                nc.vector.tensor_mul(
                    out=qsq[:], in0=qsb[:, ts(tci, CH)], in1=qsb[:, ts(tci, CH)]
                )
                sqs.append(qsq)
            nc.gpsimd.dma_start(out=q_sp[j], in_=qsb[:])
            # quad ssq MMs (col-groups 0/32/64/96), staggered one group so the
            # in-order PE queue never waits on the ACT->DVE square chain
            pending_qssq.append((j, sqs))
            if len(pending_qssq) > 1:
                jd, sq_ = pending_qssq.pop(0)
                for tci in range(NT):
                    ssq_mm(qssq, sq_[tci], tci,
                           start=(jd == 0), stop=(jd == ND - 1))
            # interleaved scan pipeline
            if j >= 1 and (j - 1) < ND:
                emit_scan(j - 1)
            if j >= SCAN_DELAY + 1:
                emit_y(j - SCAN_DELAY - 1)
        emit_scan(ND - 1)
        while pending_qssq:
            jd, sq_ = pending_qssq.pop(0)
            for tci in range(NT):
                ssq_mm(qssq, sq_[tci], tci, start=(jd == 0), stop=(jd == ND - 1))
        for c in range(max(ND - SCAN_DELAY - 1, 0), ND):
            emit_y(c)

        inv_chain(qssq, inv_q, -0.5)

        # ================= tail =================
        ql_tiles, yl_tiles = {}, {}
        for c in range(min(2, ND)):
            ql_tiles[c] = s16.tile([P, TL_], bf16, tag="s16", name=f"ql{c}")
            nc.sync.dma_start(out=ql_tiles[c][:], in_=q_sp[c])
            yl_tiles[c] = s16.tile([P, TL_], bf16, tag="s16", name=f"yl{c}")
            nc.sync.dma_start(out=yl_tiles[c][:], in_=y_sp[c])
        for c in range(ND):
            if c + 2 < ND:
                ql_tiles[c + 2] = s16.tile([P, TL_], bf16, tag="s16",
                                           name=f"ql{c + 2}")
                nc.sync.dma_start(out=ql_tiles[c + 2][:], in_=q_sp[c + 2])
                yl_tiles[c + 2] = s16.tile([P, TL_], bf16, tag="s16",
                                           name=f"yl{c + 2}")
                nc.sync.dma_start(out=yl_tiles[c + 2][:], in_=y_sp[c + 2])
            qi = s16.tile([P, TL_], bf16, tag="s16", name=f"qi{c}")
            nc.vector.tensor_mul(out=qi[:], in0=ql_tiles.pop(c)[:], in1=inv_q[:])
            sg = s16.tile([P, TL_], bf16, tag="s16", name=f"sg{c}")
            nc.scalar.activation(sg[:], qi[:], AF.Sigmoid)
            outc = s16.tile([P, TL_], bf16, tag="s16", name=f"out{c}")
            nc.vector.tensor_mul(out=outc[:], in0=sg[:], in1=yl_tiles.pop(c)[:])
            nc.gpsimd.dma_start(out=out_h[c], in_=outc[:])

    nc.finalize()
    return nc


def make_in_maps(x, w_qkv, D_=D, TL_=TL, n_cores=NCORES):
    """Host-side shard + layout prep. Returns per-core input dicts."""
    P = 128
    ND = D_ // P
    E = w_qkv.shape[0]
    n_eblk = E // P
    b_count = x.shape[0]
    halves = n_cores // b_count

    # wT tiled: [e_blk, p, do, pe] with wtile[blk, p, do, e] = w_qkv[blk*128+e, do*128+p]
    wt = (
        np.ascontiguousarray(
            w_qkv.T.reshape(ND, P, n_eblk, P).transpose(2, 1, 0, 3)
        ).astype(BF16)
    )

    in_maps = []
    for core in range(n_cores):
        b, h = divmod(core, halves)
        shard = x[b, h * TL_ : (h + 1) * TL_, :]  # [TL, D]
        xt = np.ascontiguousarray(
            shard.T.reshape(ND, P, TL_).transpose(1, 0, 2)
        ).astype(BF16)
        odd = float(h % 2 == 1)
        in_maps.append(
            {
                "xT": xt,
                "wT": wt,
                "cmask": np.full((P, 1), odd, dtype=np.float32),
                "smask": np.full((P, 1), 1.0 - odd, dtype=np.float32),
            }
        )
    return in_maps


def assemble_output(results, x, D_=D, TL_=TL, n_cores=NCORES):
    b_count = x.shape[0]
    halves = n_cores // b_count
    out2 = np.empty((b_count, halves * TL_, D_), dtype=np.float32)
    for core in range(n_cores):
        b, h = divmod(core, halves)
        outT = np.asarray(results[core]["outT"]).astype(np.float32).reshape(D_, TL_)
        out2[b, h * TL_ : (h + 1) * TL_, :] = outT.T
    return out2


_CACHED_NC = None


def kernel(x, w_qkv):
    global _CACHED_NC
    from concourse.bass_utils import run_bass_kernel_spmd

    x = np.asarray(x, dtype=np.float32)
    w_qkv = np.asarray(w_qkv, dtype=np.float32)

    if _CACHED_NC is None:
        _CACHED_NC = build_kernel()
    in_maps = make_in_maps(x, w_qkv)
    res = run_bass_kernel_spmd(_CACHED_NC, in_maps, core_ids=list(range(NCORES)))
    out2 = assemble_output(res.results, x)
    return (x, out2)


# revision 31
# speedup vs baseline: 1.2659x; 1.0339x over previous
"""AttentionFreeTransformer kernel for 8 TRN2 NeuronCores.

Reference computation (B=4, T=4096, D=2048):
    qkv = rmsnorm(x) @ w_qkv.T            # [B, T, 3D]
    q, k, v = split(qkv)
    q = rmsnorm(q); k = rmsnorm(k)
    w = exp(k); kv = w * v
    y = cumsum(kv, T) / (cumsum(w, T) + 1e-6)
    out = (x, sigmoid(q) * y)

Sharding: core = 2*b + h owns batch b, sequence half h (TL=2048 tokens).
Device tensors live transposed [channel partitions, token free] so the
T-cumsum is a DVE tensor_tensor_scan along the free axis; the cross-core
carry (first-half column totals -> second-half core) is the scan's
per-partition `initial`, exchanged with one 16KB pairwise AllReduce.

Schedule (PE never idles between phases; everything else hides under it):
  K phase   tci-outer in two j-halves with resident K weights, chasing the
            xT chunk DMAs so matmuls start ~15us in.  k ssq accumulated
            per chunk; x squares+ssq quads interleaved; k spilled to DRAM.
  V phase   j-outer streamed weights; v=psum*inv_x (DVE), kn=k*inv_k
            (gpsimd), w=exp (ACT, accum), kv=w*v (DVE stt, accum); w/kv
            spilled.  Carry AllReduce at the end, overlapped with Q.
  Q phase   j-outer streamed weights; q chunks copied+squared from psum
            (quad ssq MMs); q spilled.  Scans + ln/exp + y-mul for all 16
            channels interleaved on DVE/ACT/gpsimd under the Q matmuls,
            y spilled, then prefetch q/y reloads for the tail.
  tail      inv_q chain, then per channel sigmoid(q*inv_q)*y, bf16 out.

Algebraic notes:
  - rmsnorm(x)'s per-token scale inv_x factors out of the projection;
    q and k are re-rmsnormed which cancels it, so only v needs inv_x.
  - rsqrt/reciprocal via exp(-0.5*ln(.)) / exp(-ln(.)) on ACT
    (natural_log_exp table set; Rsqrt/Reciprocal ACT funcs banned).
"""

import sys

sys.path.insert(0, "/opt/trn_rl_repo")

import numpy as np
import ml_dtypes

import concourse.bass as bass
import concourse.bacc as bacc_mod
import concourse.mybir as mybir
from concourse.bass import ds, ts
from concourse.tile import TileContext

BF16 = ml_dtypes.bfloat16
F32EPS = float(np.finfo(np.float32).eps)

B, T, D = 4, 4096, 2048
NCORES = 8
TL = T // 2  # tokens per core

AF = mybir.ActivationFunctionType
ALU = mybir.AluOpType


class _Bacc(bacc_mod.Bacc):
    """Bacc whose act-table chooser maps all our funcs to one set.

    Forces Exp/Ln/Square/Copy -> natural_log_exp_and_others and
    Sigmoid -> sigmoid_and_others: 2 ACT_TABLE_LOADs total."""

    def insert_act_table_loads(self):
        from concourse.hw_specs import get_activation_tables
        from concourse.bacc import _bass_rust

        has_activation = any(
            isinstance(i, mybir.InstActivation)
            for b in self.main_func.blocks
            for i in b.instructions
        )
        if not has_activation:
            return
        ours = {AF.Exp, AF.Ln, AF.Square, AF.Copy, AF.Identity, AF.Sigmoid}
        tables = []
        for name, funcs in get_activation_tables(self.m.arch).items():
            if name == "natural_log_exp_and_others":
                tables.append((name, funcs))
            elif name == "sigmoid_and_others":
                tables.append((name, (funcs - ours) | {AF.Sigmoid}))
            else:
                tables.append((name, funcs - ours))
        _bass_rust.insert_act_table_loads(self, tables)


def build_kernel(D_=D, TL_=TL, n_cores=NCORES):
    P = 128
    CH = 512              # token chunk (psum free dim)
    ND = D_ // P          # channel subtiles per projection
    NT = TL_ // CH        # token chunks
    NDH = max(ND // 4, 1) # j-group size for the K phase sub-phases
    inv_scale = 1.0 / D_

    nc = _Bacc(target_bir_lowering=False, num_devices=n_cores)

    f32 = mybir.dt.float32
    bf16 = mybir.dt.bfloat16

    xT_h = nc.declare_dram_parameter("xT", [P, ND, TL_], bf16, isOutput=False)
    wT_h = nc.declare_dram_parameter("wT", [3 * ND, P, ND, P], bf16, isOutput=False)
    cmask_h = nc.declare_dram_parameter("cmask", [P, 1], f32, isOutput=False)
    smask_h = nc.declare_dram_parameter("smask", [P, 1], f32, isOutput=False)
    out_h = nc.declare_dram_parameter("outT", [ND, P, TL_], bf16, isOutput=True)

    ones_col_h = nc.inline_tensor(np.ones((P, 1), dtype=BF16), name="ones_col")
    ones_row_h = nc.inline_tensor(np.ones((1, P), dtype=BF16), name="ones_row")

    groups = [[i, i + 1] for i in range(0, n_cores, 2)]

    with (
        TileContext(nc) as tc,
        tc.tile_pool(name="const", bufs=1) as const,
        tc.tile_pool(name="wk", bufs=NDH + 1) as wkp,      # K weights (sub-phase)
        tc.tile_pool(name="wstream", bufs=3) as wstream,   # V/Q streamed weights
        tc.tile_pool(name="chunk", bufs=11) as chunkp,     # [P,CH] bf16 chunks
        tc.tile_pool(name="s16", bufs=11) as s16,          # [P,TL] bf16 scratch
        tc.tile_pool(name="lw32", bufs=1) as lw32,         # [P,TL] f32 scratch
        tc.tile_pool(name="qip", bufs=2) as qip,           # tail qi tiles
        tc.tile_pool(name="sgp", bufs=2) as sgp,           # tail sigmoid tiles
        tc.tile_pool(name="outp", bufs=2) as outp,         # tail out tiles
        tc.tile_pool(name="mmps", bufs=5, space="PSUM") as mmps,
        tc.tile_pool(name="ssqps", bufs=2, space="PSUM") as ssqps,
        tc.tile_pool(name="repps", bufs=1, space="PSUM") as repps,
        tc.tile_pool(name="spill", bufs=1, space="DRAM") as spill,
    ):
        # ---- constants / resident tiles (DMAs issued after the hot loads) ----
        ones_col = const.tile([P, 1], bf16, tag="ones_col")
        ones_row = const.tile([1, P], bf16, tag="ones_row")
        cmask = const.tile([P, 1], f32, tag="cmask")
        smask = const.tile([P, 1], f32, tag="smask")

        eps_b = const.tile([P, 1], f32, tag="eps_b")
        nc.vector.memset(eps_b[:], F32EPS)
        eps6_b = const.tile([P, 1], f32, tag="eps6_b")
        nc.vector.memset(eps6_b[:], 1e-6)

        inv_x = const.tile([P, TL_], bf16, tag="inv_x")
        inv_k = const.tile([P, TL_], bf16, tag="inv_k")
        inv_q = const.tile([P, TL_], bf16, tag="inv_q")
        carry_both = const.tile([P, 2 * ND], f32, tag="carry_both")
        carry_use = const.tile([P, 2 * ND], f32, tag="carry_use")

        # xT as chunk tiles so matmuls chase the load; chunk 0 split in two
        # half-tiles so the very first groups start on half the data.
        NDA = ND // 2 if ND > 1 else ND
        xT_c = {}
        if ND > 1:
            xT_c[(0, 0)] = const.tile([P, NDA, CH], bf16, tag="xT0a", name="xT_c0a")
            xT_c[(0, 1)] = const.tile([P, ND - NDA, CH], bf16, tag="xT0b",
                                      name="xT_c0b")
        else:
            xT_c[(0, 0)] = const.tile([P, ND, CH], bf16, tag="xT0a", name="xT_c0a")
        for tci in range(1, NT):
            xT_c[(tci, 0)] = const.tile([P, ND, CH], bf16, tag=f"xT{tci}",
                                        name=f"xT_c{tci}")

        def x_ap(tci, do):
            if tci == 0 and ND > 1:
                if do < NDA:
                    return xT_c[(0, 0)][:, do, :]
                return xT_c[(0, 1)][:, do - NDA, :]
            return xT_c[(tci, 0)][:, do, :]

        # ---- DRAM spill arrays ----
        k_sp = spill.tile([ND, P, TL_], bf16, tag="k_sp")
        q_sp = spill.tile([ND, P, TL_], bf16, tag="q_sp")
        w_sp = spill.tile([ND, P, TL_], bf16, tag="w_sp")
        kv_sp = spill.tile([ND, P, TL_], bf16, tag="kv_sp")
        y_sp = spill.tile([ND, P, TL_], bf16, tag="y_sp")
        cc_in = spill.tile([P, 2 * ND], f32, tag="cc_in")
        cc_out = spill.tile([P, 2 * ND], f32, tag="cc_out")

        # ---- input DMAs: first K weight block + xT chunk 0 first ----
        wk_sb = {}
        wk_sb[0] = wkp.tile([P, ND, P], bf16, tag="wk", name="wk0")
        nc.scalar.dma_start(out=wk_sb[0][:], in_=wT_h[ND + 0])
        nc.sync.dma_start(out=xT_c[(0, 0)][:], in_=xT_h[:, :NDA, ts(0, CH)])
        if ND > 1:
            nc.sync.dma_start(out=xT_c[(0, 1)][:], in_=xT_h[:, NDA:, ts(0, CH)])
        for j in range(1, NDH):
            wk_sb[j] = wkp.tile([P, ND, P], bf16, tag="wk", name=f"wk{j}")
            nc.sync.dma_start(out=wk_sb[j][:], in_=wT_h[ND + j])
        for tci in range(1, NT):
            nc.sync.dma_start(out=xT_c[(tci, 0)][:], in_=xT_h[:, :, ts(tci, CH)])
        nc.sync.dma_start(out=ones_col[:], in_=ones_col_h[:])
        nc.sync.dma_start(out=ones_row[:], in_=ones_row_h[:])
        nc.sync.dma_start(out=cmask[:], in_=cmask_h[:])
        nc.sync.dma_start(out=smask[:], in_=smask_h[:])

        # ssq accumulators: one [P,CH] psum tile per projection, row 32*tci
        xssq = ssqps.tile([P, CH], f32, tag="ssq", name="xssq")
        kssq = ssqps.tile([P, CH], f32, tag="ssq", name="kssq")

        def proj_group(wsb, tci, name):
            """One accumulation group: psum[P,CH] = w_blk.T @ xT chunk."""
            pk = mmps.tile([P, CH], f32, tag="mm", name=name)
            for do in range(ND):
                nc.tensor.matmul(
                    out=pk[:],
                    lhsT=wsb[:, do, :],
                    rhs=x_ap(tci, do),
                    start=(do == 0),
                    stop=(do == ND - 1),
                )
            return pk

        def ssq_mm(ssq_tile, sq_chunk, tci, start, stop):
            # explicit tile_position: auto-derive rejects base partition 96
            nc.tensor.matmul(
                out=ssq_tile[32 * tci : 32 * tci + 1, :],
                lhsT=ones_col[:],
                rhs=sq_chunk[:],
                start=start,
                stop=stop,
                tile_position=(0, 32 * tci),
            )

        def inv_chain(ssq_tile, dest, extra_scale):
            """dest[p,t] = (ssq[t]/D + eps) ** (extra_scale) replicated."""
            row = s16.tile([1, TL_], bf16, tag="s16", name="invrow")
            for tci in range(NT):
                nc.scalar.copy(
                    out=row[:, ts(tci, CH)],
                    in_=ssq_tile[32 * tci : 32 * tci + 1, :],
                )
            lnv = lw32.tile([P, TL_], f32, tag="lw")
            for tci in range(NT):
                rep = repps.tile([P, CH], f32, tag="rep", name="rep")
                nc.tensor.matmul(
                    out=rep[:],
                    lhsT=ones_row[:],
                    rhs=row[:, ts(tci, CH)],
                    start=True,
                    stop=True,
                )
                nc.scalar.activation(
                    lnv[:, ts(tci, CH)], rep[:], AF.Ln,
                    bias=eps_b[:], scale=inv_scale,
                )
            nc.scalar.activation(dest[:], lnv[:], AF.Exp, scale=extra_scale)

        # ================= K phase =================
        # two j-halves; tci-outer within each so MMs chase the xT chunks.
        # k ssq: row 32*tci accumulates over all ND j's (across both halves).
        pending_kssq = []  # staggered one group to keep PE in-order happy

        def flush_kssq(n):
            while len(pending_kssq) > n:
                pending_kssq.pop(0)()

        for jh in range(ND // NDH):
            j0 = jh * NDH
            for tci in range(NT):
                for jj in range(NDH):
                    j = j0 + jj
                    if j not in wk_sb:
                        wk_sb[j] = wkp.tile([P, ND, P], bf16, tag="wk", name=f"wk{j}")
                        nc.sync.dma_start(out=wk_sb[j][:], in_=wT_h[ND + j])
                    pk = proj_group(wk_sb[j], tci, f"pk{j}_{tci}")
                    ksb = chunkp.tile([P, CH], bf16, tag="ch", name=f"k{j}_{tci}")
                    nc.scalar.copy(out=ksb[:], in_=pk[:])
                    ksq = chunkp.tile([P, CH], bf16, tag="ch", name=f"ksq{j}_{tci}")
                    nc.vector.tensor_mul(out=ksq[:], in0=ksb[:], in1=ksb[:])
                    nc.gpsimd.dma_start(out=k_sp[j, :, ts(tci, CH)], in_=ksb[:])
                    pending_kssq.append(
                        (lambda kq=ksq, tc_=tci, j_=j: ssq_mm(
                            kssq, kq, tc_, start=(j_ == 0), stop=(j_ == ND - 1)))
                    )
                    flush_kssq(2)
        # x ssq: DVE squares + quad MMs (emitted after K proj loops;
        # DVE has slack in K and deps are long ready)
        for do in range(ND):
            sq = s16.tile([P, TL_], bf16, tag="s16", name=f"xsq{do}")
            for tci in range(NT):
                nc.vector.tensor_mul(
                    out=sq[:, ts(tci, CH)],
                    in0=x_ap(tci, do), in1=x_ap(tci, do),
                )
            for tci in range(NT):
                ssq_mm(xssq, sq[:, ts(tci, CH)], tci,
                       start=(do == 0), stop=(do == ND - 1))
        flush_kssq(0)

        inv_chain(kssq, inv_k, -0.5)
        inv_chain(xssq, inv_x, -0.5)

        # ================= V phase =================
        kc_tiles = {}
        for c in range(min(2, ND)):  # prefetch k reloads
            kc_tiles[c] = s16.tile([P, TL_], bf16, tag="s16", name=f"kc{c}")
            nc.sync.dma_start(out=kc_tiles[c][:], in_=k_sp[c])

        for c in range(ND):
            wv = wstream.tile([P, ND, P], bf16, tag="wv", name=f"wv{c}")
            nc.sync.dma_start(out=wv[:], in_=wT_h[2 * ND + c])
            vsb = s16.tile([P, TL_], bf16, tag="s16", name=f"v{c}")
            for tci in range(NT):
                pv = proj_group(wv, tci, f"pv{c}_{tci}")
                nc.vector.tensor_mul(
                    out=vsb[:, ts(tci, CH)], in0=pv[:], in1=inv_x[:, ts(tci, CH)]
                )
            if c + 2 < ND:
                kc_tiles[c + 2] = s16.tile([P, TL_], bf16, tag="s16",
                                           name=f"kc{c + 2}")
                nc.sync.dma_start(out=kc_tiles[c + 2][:], in_=k_sp[c + 2])
            kn = s16.tile([P, TL_], bf16, tag="s16", name=f"kn{c}")
            nc.gpsimd.tensor_mul(out=kn[:], in0=kc_tiles[c][:], in1=inv_k[:])
            wc = s16.tile([P, TL_], bf16, tag="s16", name=f"w{c}")
            nc.scalar.activation(
                wc[:], kn[:], AF.Exp, accum_out=carry_both[:, c : c + 1]
            )
            kvc = s16.tile([P, TL_], bf16, tag="s16", name=f"kv{c}")
            nc.vector.scalar_tensor_tensor(
                out=kvc[:], in0=wc[:], scalar=1.0, in1=vsb[:],
                op0=ALU.mult, op1=ALU.mult,
                accum_out=carry_both[:, ND + c : ND + c + 1],
            )
            nc.gpsimd.dma_start(out=w_sp[c], in_=wc[:])
            nc.gpsimd.dma_start(out=kv_sp[c], in_=kvc[:])

        # ---- carry exchange (overlaps with Q phase matmuls) ----
        snd = lw32.tile([P, 2 * ND], f32, tag="lw", name="snd")
        nc.vector.tensor_scalar_mul(snd[:], carry_both[:], smask[:])
        nc.gpsimd.dma_start(out=cc_in[:], in_=snd[:])
        nc.gpsimd.collective_compute(
            "AllReduce",
            ALU.add,
            replica_groups=groups,
            ins=[cc_in[:]],
            outs=[cc_out[:]],
        )
        rcv = lw32.tile([P, 2 * ND], f32, tag="lw", name="rcv")
        nc.sync.dma_start(out=rcv[:], in_=cc_out[:])
        nc.vector.tensor_scalar_mul(carry_use[:], rcv[:], cmask[:])

        # ================= Q phase (+ scans interleaved) =================
        qssq = ssqps.tile([P, CH], f32, tag="ssq", name="qssq")
        SCAN_DELAY = 1  # groups between a channel's scan and its y compute

        scan_tiles = {}

        def emit_scan(c):
            """Reload w/kv, scan both, compute y, spill y."""
            wld = s16.tile([P, TL_], bf16, tag="s16", name=f"wld{c}")
            nc.sync.dma_start(out=wld[:], in_=w_sp[c])
            kvld = s16.tile([P, TL_], bf16, tag="s16", name=f"kvld{c}")
            nc.sync.dma_start(out=kvld[:], in_=kv_sp[c])
            wcum = s16.tile([P, TL_], bf16, tag="s16", name=f"wcum{c}")
            nc.vector.tensor_tensor_scan(
                out=wcum[:], data0=wld[:], data1=wld[:],
                initial=carry_use[:, c : c + 1],
                op0=ALU.add, op1=ALU.bypass,
            )
            kvcum = s16.tile([P, TL_], bf16, tag="s16", name=f"kvcum{c}")
            nc.vector.tensor_tensor_scan(
                out=kvcum[:], data0=kvld[:], data1=kvld[:],
                initial=carry_use[:, ND + c : ND + c + 1],
                op0=ALU.add, op1=ALU.bypass,
            )
            scan_tiles[c] = (wcum, kvcum)

        y_resident = {}  # last channels: keep y in SBUF, skip spill+reload

        def emit_y(c):
            wcum, kvcum = scan_tiles.pop(c)
            lw = lw32.tile([P, TL_], f32, tag="lw", name=f"lw{c}")
            nc.scalar.activation(lw[:], wcum[:], AF.Ln, bias=eps6_b[:])
            rw = s16.tile([P, TL_], bf16, tag="s16", name=f"rw{c}")
            nc.scalar.activation(rw[:], lw[:], AF.Exp, scale=-1.0)
            yc = s16.tile([P, TL_], bf16, tag="s16", name=f"y{c}")
            late = c >= ND - 3
            y_eng = nc.vector if late else nc.gpsimd
            y_eng.tensor_mul(out=yc[:], in0=kvcum[:], in1=rw[:])
            if late:
                y_resident[c] = yc
            else:
                nc.gpsimd.dma_start(out=y_sp[c], in_=yc[:])

        pending_qssq = []
        for j in range(ND):
            wq = wstream.tile([P, ND, P], bf16, tag="wq", name=f"wq{j}")
            nc.sync.dma_start(out=wq[:], in_=wT_h[0 + j])
            qsb = s16.tile([P, TL_], bf16, tag="s16", name=f"q{j}")
            sqs = []
            for tci in range(NT):
                pq = proj_group(wq, tci, f"pq{j}_{tci}")
                nc.scalar.copy(out=qsb[:, ts(tci, CH)], in_=pq[:])
                qsq = chunkp.tile([P, CH], bf16, tag="ch", name=f"qsq{j}_{tci}")
                nc.vector.tensor_mul(
                    out=qsq[:], in0=qsb[:, ts(tci, CH)], in1=qsb[:, ts(tci, CH)]
                )
                sqs.append(qsq)
            nc.gpsimd.dma_start(out=q_sp[j], in_=qsb[:])
            # quad ssq MMs (col-groups 0/32/64/96), staggered one group so the
            # in-order PE queue never waits on the ACT->DVE square chain
            pending_qssq.append((j, sqs))
            if len(pending_qssq) > 2:
                jd, sq_ = pending_qssq.pop(0)
                for tci in range(NT):
                    ssq_mm(qssq, sq_[tci], tci,
                           start=(jd == 0), stop=(jd == ND - 1))
            # interleaved scan pipeline
            if j >= 1 and (j - 1) < ND:
                emit_scan(j - 1)
            if j >= SCAN_DELAY + 1:
                emit_y(j - SCAN_DELAY - 1)
        emit_scan(ND - 1)
        while pending_qssq:
            jd, sq_ = pending_qssq.pop(0)
            for tci in range(NT):
                ssq_mm(qssq, sq_[tci], tci, start=(jd == 0), stop=(jd == ND - 1))
        # inv_q chain first so its ACT row-copies beat the last y lns in queue
        inv_chain(qssq, inv_q, -0.5)
        for c in range(max(ND - SCAN_DELAY - 1, 0), ND):
            emit_y(c)

        # ================= tail =================
        ql_tiles, yl_tiles = {}, {}

        def prefetch_tail(c):
            ql_tiles[c] = s16.tile([P, TL_], bf16, tag="s16", name=f"ql{c}")
            nc.sync.dma_start(out=ql_tiles[c][:], in_=q_sp[c])
            if c not in y_resident:
                yl_tiles[c] = s16.tile([P, TL_], bf16, tag="s16", name=f"yl{c}")
                nc.sync.dma_start(out=yl_tiles[c][:], in_=y_sp[c])

        for c in range(min(2, ND)):
            prefetch_tail(c)
        for c in range(ND):
            if c + 2 < ND:
                prefetch_tail(c + 2)
            qi = qip.tile([P, TL_], bf16, tag="qi", name=f"qi{c}")
            nc.vector.tensor_mul(out=qi[:], in0=ql_tiles.pop(c)[:], in1=inv_q[:])
            sg = sgp.tile([P, TL_], bf16, tag="sg", name=f"sg{c}")
            nc.scalar.activation(sg[:], qi[:], AF.Sigmoid)
            yl = y_resident.pop(c) if c in y_resident else yl_tiles.pop(c)
            outc = outp.tile([P, TL_], bf16, tag="out", name=f"out{c}")
            nc.vector.tensor_mul(out=outc[:], in0=sg[:], in1=yl[:])
            nc.gpsimd.dma_start(out=out_h[c], in_=outc[:])

    nc.finalize()
    return nc


def make_in_maps(x, w_qkv, D_=D, TL_=TL, n_cores=NCORES):
    """Host-side shard + layout prep. Returns per-core input dicts."""
    P = 128
    ND = D_ // P
    E = w_qkv.shape[0]
    n_eblk = E // P
    b_count = x.shape[0]
    halves = n_cores // b_count

    # wT tiled: [e_blk, p, do, pe] with wtile[blk, p, do, e] = w_qkv[blk*128+e, do*128+p]
    wt = (
        np.ascontiguousarray(
            w_qkv.T.reshape(ND, P, n_eblk, P).transpose(2, 1, 0, 3)
        ).astype(BF16)
    )

    in_maps = []
    for core in range(n_cores):
        b, h = divmod(core, halves)
        shard = x[b, h * TL_ : (h + 1) * TL_, :]  # [TL, D]
        xt = np.ascontiguousarray(
            shard.T.reshape(ND, P, TL_).transpose(1, 0, 2)
        ).astype(BF16)
        odd = float(h % 2 == 1)
        in_maps.append(
            {
                "xT": xt,
                "wT": wt,
                "cmask": np.full((P, 1), odd, dtype=np.float32),
                "smask": np.full((P, 1), 1.0 - odd, dtype=np.float32),
            }
        )
    return in_maps


def assemble_output(results, x, D_=D, TL_=TL, n_cores=NCORES):
    b_count = x.shape[0]
    halves = n_cores // b_count
    out2 = np.empty((b_count, halves * TL_, D_), dtype=np.float32)
    for core in range(n_cores):
        b, h = divmod(core, halves)
        outT = np.asarray(results[core]["outT"]).astype(np.float32).reshape(D_, TL_)
        out2[b, h * TL_ : (h + 1) * TL_, :] = outT.T
    return out2


_CACHED_NC = None


def kernel(x, w_qkv):
    global _CACHED_NC
    from concourse.bass_utils import run_bass_kernel_spmd

    x = np.asarray(x, dtype=np.float32)
    w_qkv = np.asarray(w_qkv, dtype=np.float32)

    if _CACHED_NC is None:
        _CACHED_NC = build_kernel()
    in_maps = make_in_maps(x, w_qkv)
    res = run_bass_kernel_spmd(_CACHED_NC, in_maps, core_ids=list(range(NCORES)))
    out2 = assemble_output(res.results, x)
    return (x, out2)
